# revision 1
# baseline (speedup 1.0000x reference)
"""MiniCPM (MLA-style) attention — Trainium2 Bass kernel, 8-way sharded.

Strategy (per spec sharding_hint, adapted for the MLA low-rank structure):
  - Phase A (sequence-parallel): each core computes the low-rank a-path for
    its 256-row block: q_a = hs @ wq_a -> rms_norm; ckv = hs @ wkv_a ->
    rms_norm(compressed) + RoPE(k_pe). Results are transposed on-chip (PE
    transpose) and AllGathered so every core holds the full-length latent
    activations transposed: q_a_n^T [768,2048], ckv_n^T [256,2048],
    k_pe^T [32,2048].  Gathering the *latents* (low-rank!) is 8.6MB total vs
    70MB+ for gathered Q/K/V.
  - Phase B/C (tensor-parallel over heads, 5 heads/core): Q^T/K^T/V built by
    f32r matmuls directly in transposed layout; causal attention computed as
    S^T[k,q] tiles (scores transposed) so softmax-normalized probs feed the
    PV matmul as the moving operand; a ones-column appended to V yields the
    softmax denominators for free in the same matmul. RoPE on q_pe is done in
    transposed layout using an extra set of column-swapped wq_b columns so
    the "rotate_half" partner arrives in matching partitions.
  - wo: each core computes a full [2048,2560] partial with its 320 rows of
    wo; host sums the 8 partials (cheap reduction, avoids a 20MB AllReduce).

All matmuls run as float32r (FP22 multiplies, fp32 accumulate): full PE rate
with ~2.7e-4 end-to-end max relative error vs the fp32 reference (validated
in numpy simulation). Softmax skips max-subtraction: causal |scores| <= ~2.5.
"""

import sys
sys.path.insert(0, "/opt/trn_rl_repo")

from contextlib import ExitStack

import numpy as np

import concourse.bass as bass
import concourse.bacc as bacc
import concourse.tile as tile
from concourse import mybir
from concourse.bass_utils import run_bass_kernel_spmd
from concourse.masks import make_identity

F32 = mybir.dt.float32
F32R = mybir.dt.float32r
AF = mybir.ActivationFunctionType

M = 8                  # cores
S = 2048               # sequence
H = 2560               # hidden
RB = S // M            # 256 rows per core (phase A)
QLR = 768              # q low rank
CKV = 256              # kv low rank (normed part)
QK_ROPE = 32
QK_NOPE = 64
Q_HEAD = 96
V_HEAD = 64
NH = 40
NHL = NH // M          # 5 heads per core
EPS = 1e-6
SM_SCALE = float(Q_HEAD) ** -0.5
AGROWS = QLR + CKV + QK_ROPE    # 1056
NKT = S // 128         # 16 k-tiles
NQB = S // 512         # 4 q-blocks
VROW = NHL * (V_HEAD + 1)       # 325: per k-tile V' row layout (5x(64+ones))

_cache = {}


def _build():
    nc = bacc.Bacc(trn_type="TRN2", target_bir_lowering=False, debug=False,
                   num_devices=M)

    # ---- I/O ----
    hs_b = nc.dram_tensor("hs_b", [RB, H], F32, kind="ExternalInput").ap()
    cosb = nc.dram_tensor("cosb", [RB, QK_ROPE], F32, kind="ExternalInput").ap()
    ssinb = nc.dram_tensor("ssinb", [RB, QK_ROPE], F32, kind="ExternalInput").ap()
    cosT = nc.dram_tensor("cosT", [QK_ROPE, S], F32, kind="ExternalInput").ap()
    ssinT = nc.dram_tensor("ssinT", [QK_ROPE, S], F32, kind="ExternalInput").ap()
    tri = nc.dram_tensor("tri", [128, 128], F32, kind="ExternalInput").ap()
    wq_a = nc.dram_tensor("wq_a", [H, QLR], F32, kind="ExternalInput").ap()
    wkv_a = nc.dram_tensor("wkv_a", [H, CKV + QK_ROPE], F32, kind="ExternalInput").ap()
    wqb_l = nc.dram_tensor("wqb_l", [QLR, NHL * 128], mybir.dt.bfloat16,
                           kind="ExternalInput").ap()
    wkvk_l = nc.dram_tensor("wkvk_l", [CKV, NHL * QK_NOPE], F32, kind="ExternalInput").ap()
    wkvv_l = nc.dram_tensor("wkvv_l", [CKV, NHL * V_HEAD], F32, kind="ExternalInput").ap()
    wo_l = nc.dram_tensor("wo_l", [NHL * V_HEAD, H], F32, kind="ExternalInput").ap()
    out_p = nc.dram_tensor("out_p", [S, H], F32, kind="ExternalOutput").ap()

    BF16 = mybir.dt.bfloat16
    agin_kv = nc.dram_tensor("agin_kv", [CKV, RB], F32,
                             kind="Internal").ap()
    agout_kv = nc.dram_tensor("agout_kv", [M * CKV, RB], F32,
                              kind="Internal", addr_space="Shared").ap()
    agin_q = nc.dram_tensor("agin_q", [QLR + QK_ROPE, RB], BF16,
                            kind="Internal").ap()
    agout_q = nc.dram_tensor("agout_q", [M * (QLR + QK_ROPE), RB], BF16,
                             kind="Internal", addr_space="Shared").ap()
    agv_kv = agout_kv.rearrange("(r n) c -> n r c", r=M)
    agv_q = agout_q.rearrange("(r n) c -> n r c", r=M)

    with ExitStack() as ctx:
        tc = ctx.enter_context(tile.TileContext(nc))

        const = ctx.enter_context(tc.tile_pool(name="const", bufs=1))
        persist = ctx.enter_context(tc.tile_pool(name="persist", bufs=1))
        actx = ExitStack()
        sba = actx.enter_context(tc.tile_pool(name="sba", bufs=1))
        sbw = actx.enter_context(tc.tile_pool(name="sbw", bufs=2))
        ps = ctx.enter_context(tc.tile_pool(name="ps", bufs=2, space="PSUM"))

        # ---- constants ----
        ident = const.tile([128, 128], F32)
        make_identity(nc, ident)
        tri_sb = const.tile([128, 128], F32)
        nc.sync.dma_start(out=tri_sb, in_=tri)
        eps_t = const.tile([128, 1], F32)
        nc.vector.memset(eps_t, EPS)
        # packed cos/sin (transposed) [64, 2048]: rows 0:32 cosT, 32:64 ssinT
        csT = const.tile([64, S], F32)
        nc.sync.dma_start(out=csT[0:32, :], in_=cosT)
        nc.sync.dma_start(out=csT[32:64, :], in_=ssinT)
        # natural-block cos/ssin [128, 2, 32]
        csb = const.tile([128, 2, 2 * QK_ROPE], F32)
        nc.sync.dma_start(out=csb[:, :, 0:QK_ROPE],
                          in_=cosb.rearrange("(t p) c -> p t c", p=128))
        nc.sync.dma_start(out=csb[:, :, QK_ROPE:],
                          in_=ssinb.rearrange("(t p) c -> p t c", p=128))
        # b-weights resident
        wqb_sb = const.tile([128, 6, NHL * 128], mybir.dt.bfloat16)
        nc.sync.dma_start(out=wqb_sb,
                          in_=wqb_l.rearrange("(t p) c -> p t c", p=128))
        wkvk_sb = const.tile([128, 2, NHL * QK_NOPE], F32)
        nc.sync.dma_start(out=wkvk_sb.bitcast(F32R),
                          in_=wkvk_l.rearrange("(t p) c -> p t c", p=128).bitcast(F32R))
        wkvv_sb = const.tile([128, 2, NHL * V_HEAD], F32)
        nc.sync.dma_start(out=wkvv_sb.bitcast(F32R),
                          in_=wkvv_l.rearrange("(t p) c -> p t c", p=128).bitcast(F32R))

        # ---- persistent K^T and V' ----
        KT = [persist.tile([Q_HEAD, S], mybir.dt.bfloat16,
                           tag=f"KT{h}", name=f"KT{h}")
              for h in range(NHL)]
        Vp = persist.tile([128, NKT * VROW], F32, tag="Vp")
        # ones columns of V' (once)
        nc.vector.memset(Vp, 1.0)

        # ================= PHASE A =================
        hsT = []
        for rt in range(2):
            hst = sba.tile([128, H], F32, tag="hs", bufs=1, name=f"hs{rt}")
            nc.sync.dma_start(out=hst, in_=hs_b[128 * rt:128 * rt + 128, :])
            for hc in range(20):
                tp = ps.tile([128, 128], F32, tag="work")
                nc.tensor.transpose(tp, hst[:, 128 * hc:128 * hc + 128], ident)
                t = sba.tile([128, 128], F32, tag=f"hsT{rt}_{hc}",
                             name=f"hsT{rt}_{hc}")
                eng = nc.vector if (hc % 2 == 0) else nc.scalar
                if eng is nc.vector:
                    nc.vector.tensor_copy(t.bitcast(F32R), tp)
                else:
                    nc.scalar.copy(t.bitcast(F32R), tp)
                hsT.append(t)

        qa_ps = [[ps.tile([128, 384], F32, tag=f"acc{rt * 2 + jt}", bufs=1,
                          name=f"qa_ps{rt}{jt}")
                  for jt in range(2)] for rt in range(2)]
        ckv_ps = [ps.tile([128, CKV + QK_ROPE], F32, tag=f"acc{4 + rt}", bufs=1,
                          name=f"ckv_ps{rt}")
                  for rt in range(2)]
        for hp in range(10):
            wkva_t = sbw.tile([128, 2, CKV + QK_ROPE], F32, tag="wkva",
                              name=f"wkva{hp}")
            nc.scalar.dma_start(
                out=wkva_t.bitcast(F32R),
                in_=wkv_a.rearrange("(t p) c -> p t c", p=128)
                         [:, 2 * hp:2 * hp + 2, :].bitcast(F32R))
            for hh in range(2):
                hc = 2 * hp + hh
                st, sp = hc == 0, hc == 19
                for rt in range(2):
                    lhs = hsT[rt * 20 + hc].bitcast(F32R)
                    nc.tensor.matmul(ckv_ps[rt], lhs, wkva_t[:, hh, :].bitcast(F32R),
                                     start=st, stop=sp)
        for rt in range(2):
            # --- ckv rms norm on first 256 cols ---
            sq3 = sbw.tile([128, CKV], F32, tag="sq", name=f"sq3_{rt}")
            ac = sbw.tile([128, 1], F32, tag="a0", name=f"ac_{rt}")
            nc.scalar.activation(sq3, ckv_ps[rt][:, 0:CKV], AF.Square, accum_out=ac)
            nc.scalar.activation(ac, ac, AF.Sqrt, bias=eps_t, scale=1.0 / CKV)
            crstd = sbw.tile([128, 1], F32, tag="a1", name=f"crstd_{rt}")
            nc.vector.reciprocal(crstd, ac)
            ckvn = sbw.tile([128, CKV], F32, tag="ckvn", name=f"ckvn_{rt}")
            nc.vector.tensor_scalar_mul(ckvn, ckv_ps[rt][:, 0:CKV], crstd)
            for jc in range(2):
                tp = ps.tile([128, 128], F32, tag="work", name=f"tpc_{rt}_{jc}")
                nc.tensor.transpose(tp, ckvn[:, 128 * jc:128 * jc + 128], ident)
                pc = sbw.tile([128, 128], F32, tag="piece", name=f"pcc_{rt}_{jc}")
                nc.scalar.copy(pc, tp)
                nc.scalar.dma_start(out=agin_kv[128 * jc:128 * jc + 128,
                                              128 * rt:128 * rt + 128], in_=pc)
            # --- k_pe RoPE (natural) then transpose ---
            t1 = sbw.tile([128, QK_ROPE], F32, tag="kp1", name=f"kp1_{rt}")
            nc.vector.tensor_mul(t1, ckv_ps[rt][:, CKV:CKV + QK_ROPE],
                                 csb[:, rt, 0:QK_ROPE])
            t2 = sbw.tile([128, QK_ROPE], F32, tag="kp2", name=f"kp2_{rt}")
            nc.vector.tensor_mul(t2[:, 0:16], ckv_ps[rt][:, CKV + 16:CKV + 32],
                                 csb[:, rt, QK_ROPE:QK_ROPE + 16])
            nc.vector.tensor_mul(t2[:, 16:32], ckv_ps[rt][:, CKV:CKV + 16],
                                 csb[:, rt, QK_ROPE + 16:QK_ROPE + 32])
            kpe = sbw.tile([128, QK_ROPE], F32, tag="kp3", name=f"kp3_{rt}")
            nc.vector.tensor_add(kpe, t1, t2)
            tp = ps.tile([128, 128], F32, tag="work", name=f"tpk_{rt}")
            nc.tensor.transpose(tp[0:QK_ROPE, :], kpe, ident)
            pc = sbw.tile([QK_ROPE, 128], BF16, tag="pieceb", name=f"pck_{rt}")
            nc.vector.tensor_copy(pc, tp[0:QK_ROPE, :])
            nc.scalar.dma_start(out=agin_q[QLR:QLR + QK_ROPE,
                                           128 * rt:128 * rt + 128], in_=pc)

        nc.gpsimd.collective_compute(
            "AllGather", mybir.AluOpType.bypass,
            replica_groups=[list(range(M))],
            ins=[agin_kv], outs=[agout_kv],
        )

        for hp in range(10):
            wqa_t = sbw.tile([128, 2, QLR], F32, tag="wqa", name=f"wqa{hp}")
            (nc.sync if hp % 2 == 0 else nc.scalar).dma_start(
                out=wqa_t.bitcast(F32R),
                in_=wq_a.rearrange("(t p) c -> p t c", p=128)
                        [:, 2 * hp:2 * hp + 2, :].bitcast(F32R))
            for hh in range(2):
                hc = 2 * hp + hh
                st, sp = hc == 0, hc == 19
                for rt in range(2):
                    lhs = hsT[rt * 20 + hc].bitcast(F32R)
                    for jt in range(2):
                        nc.tensor.matmul(
                            qa_ps[rt][jt], lhs,
                            wqa_t[:, hh, 384 * jt:384 * jt + 384].bitcast(F32R),
                            start=st, stop=sp)

        for rt in range(2):
            # --- q_a rms norm (natural layout) ---
            sq = sbw.tile([128, 384], F32, tag="sq", name=f"sq_{rt}")
            a0 = sbw.tile([128, 1], F32, tag="a0", name=f"a0_{rt}")
            a1 = sbw.tile([128, 1], F32, tag="a1", name=f"a1_{rt}")
            nc.scalar.activation(sq, qa_ps[rt][0], AF.Square, accum_out=a0)
            sq2 = sbw.tile([128, 384], F32, tag="sq", name=f"sq2_{rt}")
            nc.scalar.activation(sq2, qa_ps[rt][1], AF.Square, accum_out=a1)
            ssum = sbw.tile([128, 1], F32, tag="a0", name=f"ssum_{rt}")
            nc.vector.tensor_add(ssum, a0, a1)
            nc.scalar.activation(ssum, ssum, AF.Sqrt, bias=eps_t, scale=1.0 / QLR)
            rstd = sbw.tile([128, 1], F32, tag="a1", name=f"rstd_{rt}")
            nc.vector.reciprocal(rstd, ssum)
            qan = sbw.tile([128, QLR], F32, tag="qan", bufs=1, name=f"qan_{rt}")
            for jt in range(2):
                nc.vector.tensor_scalar_mul(qan[:, 384 * jt:384 * jt + 384],
                                            qa_ps[rt][jt], rstd)
            for jc in range(6):
                tp = ps.tile([128, 128], F32, tag="work", name=f"tpq_{rt}_{jc}")
                nc.tensor.transpose(tp, qan[:, 128 * jc:128 * jc + 128], ident)
                pc = sbw.tile([128, 128], BF16, tag="pieceb",
                              name=f"pcq_{rt}_{jc}")
                if jc % 2 == 0:
                    nc.vector.tensor_copy(pc, tp)
                else:
                    nc.scalar.copy(pc, tp)
                nc.scalar.dma_start(out=agin_q[128 * jc:128 * jc + 128,
                                             128 * rt:128 * rt + 128], in_=pc)

        nc.gpsimd.collective_compute(
            "AllGather", mybir.AluOpType.bypass,
            replica_groups=[list(range(M))],
            ins=[agin_q], outs=[agout_q],
        )
        actx.close()
        sbc = ctx.enter_context(tc.tile_pool(name="sbc", bufs=2))

        # ================= PHASE B: K^T and V' =================
        for kb in range(NQB):
            ckt = sbc.tile([128, 2, 2, RB], F32, tag="latB", bufs=2,
                           name=f"ckt{kb}")
            for d in range(2):
                nc.sync.dma_start(
                    out=ckt[:, d].bitcast(F32R),
                    in_=agv_kv[128 * d:128 * d + 128,
                               2 * kb:2 * kb + 2, :].bitcast(F32R))
            cks = [ckt[:, 0], ckt[:, 1]]
            for h in range(NHL):
                kps = ps.tile([QK_NOPE, 512], F32, tag="work")
                for c in range(2):
                    nc.tensor.matmul(
                        kps,
                        wkvk_sb[:, c, QK_NOPE * h:QK_NOPE * h + QK_NOPE].bitcast(F32R),
                        cks[c].rearrange("p r c -> p (r c)").bitcast(F32R),
                        start=(c == 0), stop=(c == 1))
                nc.scalar.copy(KT[h][0:QK_NOPE, 512 * kb:512 * kb + 512], kps)
                nc.gpsimd.dma_start(
                    out=KT[h][QK_NOPE:Q_HEAD, 512 * kb:512 * kb + 512]
                        .rearrange("p (r c) -> p r c", r=2),
                    in_=agv_q[QLR:QLR + QK_ROPE, 2 * kb:2 * kb + 2, :])
            for t4 in range(4):
                vps = ps.tile([128, NHL * V_HEAD], F32, tag="work")
                for c in range(2):
                    nc.tensor.matmul(
                        vps,
                        cks[c].rearrange("p r c -> p (r c)")
                              [:, 128 * t4:128 * t4 + 128].bitcast(F32R),
                        wkvv_sb[:, c, :].bitcast(F32R),
                        start=(c == 0), stop=(c == 1))
                kt = 4 * kb + t4
                vdst = bass.AP(tensor=Vp.tensor,
                               offset=Vp.offset + kt * VROW,
                               ap=[Vp.ap[0], [V_HEAD + 1, NHL], [1, V_HEAD]])
                nc.vector.tensor_copy(vdst.bitcast(F32R), vps)


        # ================= PHASE C: per q-block =================
        QTs = {}
        LATs = {}

        def make_lat(qb):
            latt = sbc.tile([128, 6, 2, RB], BF16, tag="latC", bufs=2,
                            name=f"latt{qb}")
            for r in range(2):
                (nc.sync if r == 0 else nc.scalar).dma_start(
                    out=latt[:, :, r, :],
                    in_=agv_q[0:QLR].rearrange("(d p) r c -> p d r c", p=128)
                        [:, :, 2 * qb + r, :])
            LATs[qb] = [latt[:, c] for c in range(6)]
            QTs[qb] = []

        def make_qt_head(qb, h):
            qs = slice(512 * qb, 512 * qb + 512)
            lats = LATs[qb]
            wi = _wk[0]; _wk[0] += 1
            qps = ps.tile([128, 512], F32,
                          tag=("work" if wi % 3 < 2 else "acc5"),
                          bufs=(2 if wi % 3 < 2 else 1),
                          name=f"qps{qb}_{h}")
            for c in range(6):
                nc.tensor.matmul(
                    qps, wqb_sb[:, c, 128 * h:128 * h + 128],
                    lats[c].rearrange("p r c -> p (r c)"),
                    start=(c == 0), stop=(c == 5))
            qt = sbc.tile([Q_HEAD, 512], BF16, tag="QT", bufs=10,
                          name=f"qt{qb}_{h}")
            nc.scalar.copy(qt[0:QK_NOPE, :], qps[0:QK_NOPE, :])
            t1 = sbc.tile([QK_ROPE, 512], F32, tag="rp1", bufs=2,
                          name=f"rp1_{qb}_{h}")
            nc.vector.tensor_mul(t1, qps[64:96, :], csT[0:32, qs])
            t2 = sbc.tile([QK_ROPE, 512], F32, tag="rp2", bufs=2,
                          name=f"rp2_{qb}_{h}")
            nc.vector.tensor_mul(t2, qps[96:128, :], csT[32:64, qs])
            nc.vector.tensor_add(qt[QK_NOPE:Q_HEAD, :], t1, t2)
            QTs[qb].append(qt)

        def make_qt(qb):
            make_lat(qb)
            for h in range(NHL):
                make_qt_head(qb, h)

        _wk = [0]
        make_qt(0)
        for qb in range(NQB):
            qs = slice(512 * qb, 512 * qb + 512)
            QT = QTs[qb]
            att = [ps.tile([VROW // NHL, 512], F32, tag=f"acc{h}", bufs=1,
                           name=f"att{h}")
                   for h in range(NHL)]
            nkt = 4 * qb + 4
            for kt in range(nkt):
                o = max(0, 128 * kt - 512 * qb)
                for h in range(NHL):
                    wi = _wk[0]; _wk[0] += 1
                    sps = ps.tile([128, 512], F32,
                                  tag=("work" if wi % 3 < 2 else "acc5"),
                                  bufs=(2 if wi % 3 < 2 else 1),
                                  name=f"sps{qb}_{kt}_{h}")
                    nc.tensor.matmul(sps[:, o:512],
                                     KT[h][:, 128 * kt:128 * kt + 128],
                                     QT[h][:, o:512],
                                     start=True, stop=True)
                    pt = sbc.tile([128, 512], F32, tag="PT", bufs=4,
                                  name=f"pt{qb}_{kt}_{h}")
                    nc.scalar.activation(pt[:, o:512].bitcast(F32R), sps[:, o:512],
                                         AF.Exp, scale=SM_SCALE)
                    if 128 * kt >= 512 * qb:
                        nc.vector.tensor_mul(pt[:, o:o + 128].bitcast(F32R),
                                             pt[:, o:o + 128], tri_sb)
                    nc.tensor.matmul(att[h][:, o:512],
                                     Vp[:, kt * VROW + 65 * h:kt * VROW + 65 * h + 65]
                                     .bitcast(F32R),
                                     pt[:, o:512].bitcast(F32R),
                                     start=(kt == 0), stop=(kt == nkt - 1),
                                     skip_group_check=True)

            if qb + 1 < NQB:
                make_lat(qb + 1)

            aT = [sbc.tile([128, 512], F32, tag=f"aT{p}", bufs=2, name=f"aT{p}")
                  for p in range(2)]
            aT4 = sbc.tile([QK_NOPE, 512], F32, tag="aT4", bufs=2)
            for h in range(NHL):
                rc = sbc.tile([1, 512], F32, tag="rc", bufs=2,
                              name=f"rc{qb}_{h}")
                nc.vector.reciprocal(rc, att[h][64:65, :])
                bcst = sbc.tile([QK_NOPE, 512], F32, tag="bc", bufs=2,
                                name=f"bc{qb}_{h}")
                nc.gpsimd.partition_broadcast(bcst, rc)
                if h < 4:
                    dst = aT[h // 2][64 * (h % 2):64 * (h % 2) + 64, :]
                else:
                    dst = aT4
                nc.vector.tensor_mul(dst.bitcast(F32R), att[h][0:64, :], bcst)
                if qb + 1 < NQB:
                    make_qt_head(qb + 1, h)
                # HAM warm-keeper: a small matmul chained on this head's
                # broadcast so it lands mid-epilogue, keeping the PE clock
                # gate open through the DVE/GpSimd stretch.
                wi = _wk[0]; _wk[0] += 1
                wm = ps.tile([128, 512], F32,
                             tag=("work" if wi % 3 < 2 else "acc5"),
                             bufs=(2 if wi % 3 < 2 else 1),
                             name=f"warm{qb}_{h}")
                nc.tensor.matmul(wm, dst[:, 0:128].bitcast(F32R),
                                 dst.bitcast(F32R), start=True, stop=True)
            for hc in range(5):
                hcs = slice(512 * hc, 512 * hc + 512)
                w01 = sbc.tile([128, 2, 512], F32, tag="wo0", bufs=2,
                               name=f"w01_{qb}_{hc}")
                nc.gpsimd.dma_start(
                    out=w01.bitcast(F32R),
                    in_=wo_l[0:256].rearrange("(d p) c -> p d c", p=128)
                            [:, :, hcs].bitcast(F32R))
                w0, w1 = w01[:, 0], w01[:, 1]
                w2 = sbc.tile([QK_NOPE, 512], F32, tag="wo2", bufs=2,
                              name=f"w2_{qb}_{hc}")
                nc.scalar.dma_start(out=w2.bitcast(F32R),
                                  in_=wo_l[256:320, hcs].bitcast(F32R))
                for half in range(2):
                    osb = sbc.tile([128, 2, 512], F32, tag="osb", bufs=2,
                                   name=f"osb{qb}_{hc}_{half}")
                    for qq in range(2):
                        qt4 = 2 * half + qq
                        qsl = slice(128 * qt4, 128 * qt4 + 128)
                        wi = _wk[0]; _wk[0] += 1
                        ops = ps.tile([128, 512], F32,
                                      tag=("work" if wi % 3 < 2 else "acc5"),
                                      bufs=(2 if wi % 3 < 2 else 1),
                                      name=f"ops{qb}_{hc}_{qt4}")
                        nc.tensor.matmul(ops, aT[0][:, qsl].bitcast(F32R),
                                         w0.bitcast(F32R), start=True, stop=False)
                        nc.tensor.matmul(ops, aT[1][:, qsl].bitcast(F32R),
                                         w1.bitcast(F32R), start=False, stop=False)
                        nc.tensor.matmul(ops, aT4[:, qsl].bitcast(F32R),
                                         w2.bitcast(F32R), start=False, stop=True)
                        if qt4 % 2 == 0:
                            nc.vector.tensor_copy(osb[:, qq, :], ops)
                        else:
                            nc.scalar.copy(osb[:, qq, :], ops)
                    (nc.sync if hc % 2 == 0 else nc.gpsimd).dma_start(
                        out=out_p.rearrange("(d p) c -> p d c", p=128)
                                 [:, 4 * qb + 2 * half:4 * qb + 2 * half + 2, hcs],
                        in_=osb)

    nc.compile()
    return nc


def _prep(inputs):
    hs = np.ascontiguousarray(np.asarray(inputs["hidden_states"], np.float32)[0])
    cos = np.asarray(inputs["cos"], np.float32)
    sin = np.asarray(inputs["sin"], np.float32)
    wq_a = np.asarray(inputs["wq_a"], np.float32)
    q_ln = np.asarray(inputs["q_a_ln_w"], np.float32)
    wq_b = np.asarray(inputs["wq_b"], np.float32)
    wkv_a = np.asarray(inputs["wkv_a"], np.float32)
    kv_ln = np.asarray(inputs["kv_a_ln_w"], np.float32)
    wkv_b = np.asarray(inputs["wkv_b"], np.float32)
    wo = np.asarray(inputs["wo"], np.float32)

    if not np.all(q_ln == 1.0):
        wq_b = wq_b * q_ln[:, None]
    if not np.all(kv_ln == 1.0):
        wkv_b = wkv_b * kv_ln[:, None]

    ssin = np.concatenate([-sin[:, :16], sin[:, 16:]], axis=1)
    cosT = np.ascontiguousarray(cos.T)
    ssinT = np.ascontiguousarray(ssin.T)
    tri = np.triu(np.ones((128, 128), np.float32))

    in_maps = []
    for c in range(M):
        heads = range(NHL * c, NHL * c + NHL)
        qb_cols = []
        for h in heads:
            qb_cols.extend(range(96 * h, 96 * h + 96))
            # swapped pe columns: [16:32] then [0:16] of the pe block
            qb_cols.extend(range(96 * h + 80, 96 * h + 96))
            qb_cols.extend(range(96 * h + 64, 96 * h + 80))
        import ml_dtypes
        wqb_loc = np.ascontiguousarray(wq_b[:, qb_cols]).astype(ml_dtypes.bfloat16)
        kcols, vcols = [], []
        for h in heads:
            kcols.extend(range(128 * h, 128 * h + 64))
            vcols.extend(range(128 * h + 64, 128 * h + 128))
        in_maps.append({
            "hs_b": np.ascontiguousarray(hs[RB * c:RB * c + RB]),
            "cosb": np.ascontiguousarray(cos[RB * c:RB * c + RB]),
            "ssinb": np.ascontiguousarray(ssin[RB * c:RB * c + RB]),
            "cosT": cosT,
            "ssinT": ssinT,
            "tri": tri,
            "wq_a": wq_a,
            "wkv_a": wkv_a,
            "wqb_l": wqb_loc,
            "wkvk_l": np.ascontiguousarray(wkv_b[:, kcols]),
            "wkvv_l": np.ascontiguousarray(wkv_b[:, vcols]),
            "wo_l": np.ascontiguousarray(wo[NHL * V_HEAD * c:NHL * V_HEAD * (c + 1)]),
        })
    return in_maps


def kernel(**inputs):
    if "nc" not in _cache:
        _cache["nc"] = _build()
    nc = _cache["nc"]
    in_maps = _prep(inputs)
    res = run_bass_kernel_spmd(nc, in_maps, core_ids=list(range(M)))
    out = res.results[0]["out_p"].astype(np.float32)
    for c in range(1, M):
        out += res.results[c]["out_p"]
    return out.reshape(1, S, H)



# revision 12
# speedup vs baseline: 1.3393x; 1.3393x over previous
"""MiniCPM (MLA-style) attention — Trainium2 Bass kernel, 8-way sharded.

Strategy (tensor-parallel over heads, 5 heads/core; seq-parallel low-rank
phase A with AllGather of the latents):

  Phase A (seq-parallel, 256 rows/core): hs^T arrives pre-transposed from
  the host (bf16); qa/ckv computed in natural layout via bf16 matmuls with
  all weights bulk-DMAed up front; rms_norm; outputs transposed on-chip and
  AllGathered (kv latents f32, q latents + roped k_pe bf16).

  Phase B: K^T (bf16) and V' (f32) built per head from the gathered
  latents.  V' layout per k-tile: [V_h0|..|V_h4|ones64] — the 64 ones
  columns make every PV matmul emit the softmax denominator replicated
  across partitions 64:128 of the attention accumulator, so the epilogue
  is a single DVE divide (no reciprocal / partition-broadcast chain).

  Phase C (per q-block of 512, per head sequentially): scores S^T[k,q]
  computed two k-tiles at a time into a 2-bank PSUM pair, one ACT exp per
  pair (halves ACT instruction overhead), causal tri-mask on DVE, PV
  accumulation with the ones-block stationary.  wo matmuls of the previous
  q-block and Q^T builds of the next q-block are interleaved as PE filler
  so the PE never idles long enough for the HAM clock-gate to re-throttle.

  wo: resident in SBUF; each core computes a full [2048,2560] partial with
  its 320 rows; host sums the 8 partials.
"""

import sys
sys.path.insert(0, "/opt/trn_rl_repo")

from contextlib import ExitStack
from functools import partial

import numpy as np

import concourse.bass as bass
import concourse.bacc as bacc
import concourse.tile as tile
from concourse import mybir
from concourse.bass_utils import run_bass_kernel_spmd
from concourse.masks import make_identity

F32 = mybir.dt.float32
F32R = mybir.dt.float32r
BF16 = mybir.dt.bfloat16
AF = mybir.ActivationFunctionType
ALU = mybir.AluOpType

M = 8                  # cores
S = 2048               # sequence
H = 2560               # hidden
RB = S // M            # 256 rows per core (phase A)
QLR = 768              # q low rank
CKV = 256              # kv low rank (normed part)
QK_ROPE = 32
QK_NOPE = 64
Q_HEAD = 96
V_HEAD = 64
NH = 40
NHL = NH // M          # 5 heads per core
EPS = 1e-6
SM_SCALE = float(Q_HEAD) ** -0.5
NQB = S // 512         # 4 q-blocks
NKT = S // 128         # 16 k-tiles
# V' per k-tile: [ones|V0 | ones|V1 | ... | ones|V4] — every head h reads a
# CONTIGUOUS 128-col stationary window [ones64|V_h] at 128h, so the softmax
# denominator always lands partition-0-aligned in the attention accumulator
# (reciprocal_approx_fast requires a partition-0-based input).
VROW = 640
HC = H // 128              # 20 hidden-dim k-tiles

_cache = {}


def _build():
    nc = bacc.Bacc(trn_type="TRN2", target_bir_lowering=False, debug=False,
                   num_devices=M)

    # ---- I/O ----
    hsT_d = nc.dram_tensor("hsT", [H, RB], BF16, kind="ExternalInput").ap()
    wqa_d = nc.dram_tensor("wqa", [H, QLR], BF16, kind="ExternalInput").ap()
    wkva_d = nc.dram_tensor("wkva", [H, CKV + QK_ROPE], BF16,
                            kind="ExternalInput").ap()
    cosb = nc.dram_tensor("cosb", [RB, QK_ROPE], F32, kind="ExternalInput").ap()
    ssinb = nc.dram_tensor("ssinb", [RB, QK_ROPE], F32, kind="ExternalInput").ap()
    cosT = nc.dram_tensor("cosT", [QK_ROPE, S], F32, kind="ExternalInput").ap()
    ssinT = nc.dram_tensor("ssinT", [QK_ROPE, S], F32, kind="ExternalInput").ap()
    tri = nc.dram_tensor("tri", [128, 128], F32, kind="ExternalInput").ap()
    wqb_l = nc.dram_tensor("wqb_l", [QLR, NHL * 128], BF16,
                           kind="ExternalInput").ap()
    wkvk_l = nc.dram_tensor("wkvk_l", [CKV, NHL * QK_NOPE], F32,
                            kind="ExternalInput").ap()
    wkvv_l = nc.dram_tensor("wkvv_l", [CKV, NHL * V_HEAD], F32,
                            kind="ExternalInput").ap()
    wo_l = nc.dram_tensor("wo_l", [NHL * V_HEAD, H], F32,
                          kind="ExternalInput").ap()
    out_p = nc.dram_tensor("out_p", [S, H], F32, kind="ExternalOutput").ap()

    agin_kv = nc.dram_tensor("agin_kv", [CKV, RB], F32, kind="Internal").ap()
    agout_kv = nc.dram_tensor("agout_kv", [M * CKV, RB], F32,
                              kind="Internal", addr_space="Shared").ap()
    agin_q = nc.dram_tensor("agin_q", [QLR + QK_ROPE, RB], BF16,
                            kind="Internal").ap()
    agout_q = nc.dram_tensor("agout_q", [M * (QLR + QK_ROPE), RB], BF16,
                             kind="Internal", addr_space="Shared").ap()
    agv_kv = agout_kv.rearrange("(r n) c -> n r c", r=M)   # [256, 8, 256]
    agv_q = agout_q.rearrange("(r n) c -> n r c", r=M)     # [800, 8, 256]

    with ExitStack() as ctx:
        tc = ctx.enter_context(tile.TileContext(nc))

        const = ctx.enter_context(tc.tile_pool(name="const", bufs=1))
        persist = ctx.enter_context(tc.tile_pool(name="persist", bufs=1))
        ps = ctx.enter_context(tc.tile_pool(name="ps", bufs=1, space="PSUM"))
        actx = ExitStack()
        sba = actx.enter_context(tc.tile_pool(name="sba", bufs=1))

        # ---- upfront DMAs: phase-A criticals first ----
        hsT_sb = sba.tile([128, HC, RB], BF16)
        nc.sync.dma_start(out=hsT_sb, in_=hsT_d.rearrange("(t p) c -> p t c", p=128))
        wkva_sb = sba.tile([128, HC, CKV + QK_ROPE], BF16)
        nc.scalar.dma_start(out=wkva_sb,
                            in_=wkva_d.rearrange("(t p) c -> p t c", p=128))
        wqa_sb = sba.tile([128, HC, QLR], BF16)
        nc.sync.dma_start(out=wqa_sb,
                          in_=wqa_d.rearrange("(t p) c -> p t c", p=128))
        wqb_sb = const.tile([128, 6, NHL * 128], BF16)
        nc.scalar.dma_start(out=wqb_sb,
                            in_=wqb_l.rearrange("(t p) c -> p t c", p=128))

        ident = const.tile([128, 128], F32)
        make_identity(nc, ident)
        tri_sb = const.tile([128, 128], F32)
        nc.gpsimd.dma_start(out=tri_sb, in_=tri)
        eps_t = const.tile([128, 1], F32)
        nc.vector.memset(eps_t, EPS)
        # packed cos/sin (transposed) [64, 2048]: rows 0:32 cosT, 32:64 ssinT
        csT = const.tile([64, S], F32)
        nc.gpsimd.dma_start(out=csT[0:32, :], in_=cosT)
        nc.gpsimd.dma_start(out=csT[32:64, :], in_=ssinT)
        # natural-block cos/ssin [128, 2, 64]
        csb = const.tile([128, 2, 2 * QK_ROPE], F32)
        nc.gpsimd.dma_start(out=csb[:, :, 0:QK_ROPE],
                            in_=cosb.rearrange("(t p) c -> p t c", p=128))
        nc.gpsimd.dma_start(out=csb[:, :, QK_ROPE:],
                            in_=ssinb.rearrange("(t p) c -> p t c", p=128))
        wkvk_sb = const.tile([128, 2, NHL * QK_NOPE], F32)
        nc.gpsimd.dma_start(out=wkvk_sb.bitcast(F32R),
                            in_=wkvk_l.rearrange("(t p) c -> p t c", p=128).bitcast(F32R))
        wkvv_sb = const.tile([128, 2, NHL * V_HEAD], F32)
        nc.gpsimd.dma_start(out=wkvv_sb.bitcast(F32R),
                            in_=wkvv_l.rearrange("(t p) c -> p t c", p=128).bitcast(F32R))
        # wo resident (needed only in phase C — queued last)
        w01_sb = const.tile([128, 2, H], F32)
        nc.sync.dma_start(out=w01_sb.bitcast(F32R),
                          in_=wo_l[0:256].rearrange("(d p) c -> p d c", p=128)
                          .bitcast(F32R))
        w2_sb = const.tile([QK_NOPE, H], F32)
        nc.scalar.dma_start(out=w2_sb.bitcast(F32R),
                            in_=wo_l[256:320].bitcast(F32R))

        # ---- persistent K^T / V' ----
        KT = [persist.tile([128, S], BF16, tag=f"KT{h}", name=f"KT{h}")
              for h in range(NHL)]
        Vp = persist.tile([128, NKT, VROW], F32, tag="Vp")
        nc.vector.memset(Vp, 1.0)

        # ================= PHASE A =================
        # ckv = hs @ wkv_a   (rt0 in cols 0:288, rt1 in cols 512:800)
        ckv_ps = ps.tile([128, 1024], F32, tag="pr", bufs=2, name="ckv_ps")
        ckv_v = [ckv_ps[:, 0:CKV + QK_ROPE],
                 ckv_ps[:, 512:512 + CKV + QK_ROPE]]
        for hc in range(HC):
            for rt in range(2):
                nc.tensor.matmul(ckv_v[rt],
                                 hsT_sb[:, hc, 128 * rt:128 * rt + 128],
                                 wkva_sb[:, hc, :],
                                 start=(hc == 0), stop=(hc == HC - 1))
        for rt in range(2):
            sq3 = sba.tile([128, CKV], F32, tag="sq", name=f"sq3_{rt}")
            ac = sba.tile([128, 1], F32, tag="st0", name=f"ac_{rt}")
            nc.scalar.activation(sq3, ckv_v[rt][:, 0:CKV], AF.Square, accum_out=ac)
            nc.scalar.activation(ac, ac, AF.Sqrt, bias=eps_t, scale=1.0 / CKV)
            crstd = sba.tile([128, 1], F32, tag="st1", name=f"crstd_{rt}")
            nc.vector.reciprocal_approx_fast(crstd, ac)
            ckvn = sba.tile([128, CKV], F32, tag="ckvn", bufs=2, name=f"ckvn_{rt}")
            nc.vector.tensor_scalar_mul(ckvn, ckv_v[rt][:, 0:CKV], crstd)
            for jc in range(2):
                tp = ps.tile([128, 128], F32, tag=("a2" if jc == 0 else "a3"),
                             name=f"tpc_{rt}_{jc}")
                nc.tensor.transpose(tp, ckvn[:, 128 * jc:128 * jc + 128], ident)
                pc = sba.tile([128, 128], F32, tag="pc", bufs=4,
                              name=f"pcc_{rt}_{jc}")
                if jc == 0:
                    nc.vector.tensor_copy(pc, tp)
                else:
                    nc.scalar.copy(pc, tp)
                nc.scalar.dma_start(out=agin_kv[128 * jc:128 * jc + 128,
                                                128 * rt:128 * rt + 128], in_=pc)

        nc.gpsimd.collective_compute(
            "AllGather", mybir.AluOpType.bypass,
            replica_groups=[list(range(M))],
            ins=[agin_kv], outs=[agout_kv],
        )

        # k_pe RoPE (natural layout) then transpose, into agin_q rows 768:800
        for rt in range(2):
            t1 = sba.tile([128, QK_ROPE], F32, tag="kp1", name=f"kp1_{rt}")
            nc.vector.tensor_mul(t1, ckv_v[rt][:, CKV:CKV + QK_ROPE],
                                 csb[:, rt, 0:QK_ROPE])
            t2 = sba.tile([128, QK_ROPE], F32, tag="kp2", name=f"kp2_{rt}")
            nc.vector.tensor_mul(t2[:, 0:16], ckv_v[rt][:, CKV + 16:CKV + 32],
                                 csb[:, rt, QK_ROPE:QK_ROPE + 16])
            nc.vector.tensor_mul(t2[:, 16:32], ckv_v[rt][:, CKV:CKV + 16],
                                 csb[:, rt, QK_ROPE + 16:QK_ROPE + 32])
            kpe = sba.tile([128, QK_ROPE], F32, tag="kp3", name=f"kp3_{rt}")
            nc.vector.tensor_add(kpe, t1, t2)
            tp = ps.tile([128, 128], F32, tag="a2", name=f"tpk_{rt}")
            nc.tensor.transpose(tp[0:QK_ROPE, :], kpe, ident)
            pck = sba.tile([QK_ROPE, 128], BF16, tag="pck", bufs=2,
                           name=f"pck_{rt}")
            nc.vector.tensor_copy(pck, tp[0:QK_ROPE, :])
            nc.scalar.dma_start(out=agin_q[QLR:QLR + QK_ROPE,
                                           128 * rt:128 * rt + 128], in_=pck)

        # qa = hs @ wq_a  (rt0 in pr slot halves; rt1 in a0/a1)
        qa0 = ps.tile([128, 1024], F32, tag="pr", bufs=2, name="qa0")
        qa1a = ps.tile([128, 384], F32, tag="a0", name="qa1a")
        qa1b = ps.tile([128, 384], F32, tag="a1", name="qa1b")
        qa_v = [[qa0[:, 0:384], qa0[:, 512:896]], [qa1a, qa1b]]
        for hc in range(HC):
            for rt in range(2):
                for jt in range(2):
                    nc.tensor.matmul(qa_v[rt][jt],
                                     hsT_sb[:, hc, 128 * rt:128 * rt + 128],
                                     wqa_sb[:, hc, 384 * jt:384 * jt + 384],
                                     start=(hc == 0), stop=(hc == HC - 1))
        for rt in range(2):
            sq = sba.tile([128, 384], F32, tag="sq", name=f"sq_{rt}")
            a0s = sba.tile([128, 1], F32, tag="st0", name=f"a0s_{rt}")
            a1s = sba.tile([128, 1], F32, tag="st1", name=f"a1s_{rt}")
            nc.scalar.activation(sq, qa_v[rt][0], AF.Square, accum_out=a0s)
            sq2 = sba.tile([128, 384], F32, tag="sq", name=f"sq2_{rt}")
            nc.scalar.activation(sq2, qa_v[rt][1], AF.Square, accum_out=a1s)
            ssum = sba.tile([128, 1], F32, tag="st2", name=f"ssum_{rt}")
            nc.vector.tensor_add(ssum, a0s, a1s)
            nc.scalar.activation(ssum, ssum, AF.Sqrt, bias=eps_t, scale=1.0 / QLR)
            rstd = sba.tile([128, 1], F32, tag="st3", name=f"rstd_{rt}")
            nc.vector.reciprocal_approx_fast(rstd, ssum)
            qan = sba.tile([128, QLR], F32, tag="qan", bufs=2, name=f"qan_{rt}")
            for jt in range(2):
                nc.vector.tensor_scalar_mul(qan[:, 384 * jt:384 * jt + 384],
                                            qa_v[rt][jt], rstd)
            for jc in range(6):
                tp = ps.tile([128, 128], F32, tag=("a2" if jc % 2 == 0 else "a3"),
                             name=f"tpq_{rt}_{jc}")
                nc.tensor.transpose(tp, qan[:, 128 * jc:128 * jc + 128], ident)
                pcq = sba.tile([128, 128], BF16, tag="pcb", bufs=4,
                               name=f"pcq_{rt}_{jc}")
                if jc % 2 == 0:
                    nc.vector.tensor_copy(pcq, tp)
                else:
                    nc.scalar.copy(pcq, tp)
                (nc.scalar if jc % 2 == 0 else nc.sync).dma_start(
                    out=agin_q[128 * jc:128 * jc + 128,
                               128 * rt:128 * rt + 128], in_=pcq)

        nc.gpsimd.collective_compute(
            "AllGather", mybir.AluOpType.bypass,
            replica_groups=[list(range(M))],
            ins=[agin_q], outs=[agout_q],
        )
        actx.close()
        sbc = ctx.enter_context(tc.tile_pool(name="sbc", bufs=1))

        # ================= PHASE B: K^T and V' =================
        cp_engines = [nc.vector, nc.scalar]
        cpi = [0]

        def rot_copy(dst, src):
            e = cp_engines[cpi[0] % 2]
            cpi[0] += 1
            if e is nc.scalar:
                e.copy(dst, src)
            else:
                e.tensor_copy(dst, src)

        for kb in range(NQB):
            ckt = sbc.tile([128, 2, 2, RB], F32, tag="ckt", bufs=2,
                           name=f"ckt{kb}")
            for d in range(2):
                nc.sync.dma_start(
                    out=ckt[:, d].bitcast(F32R),
                    in_=agv_kv[128 * d:128 * d + 128,
                               2 * kb:2 * kb + 2, :].bitcast(F32R))
            cks = [ckt[:, c].rearrange("p r c -> p (r c)") for c in range(2)]
            for h in range(NHL):
                kps = ps.tile([128, 512], F32, tag="a2", name=f"kps{kb}_{h}")
                for c in range(2):
                    nc.tensor.matmul(
                        kps[0:QK_NOPE, :],
                        wkvk_sb[:, c, QK_NOPE * h:QK_NOPE * h + QK_NOPE]
                        .bitcast(F32R),
                        cks[c].bitcast(F32R),
                        start=(c == 0), stop=(c == 1))
                rot_copy(KT[h][0:QK_NOPE, 512 * kb:512 * kb + 512],
                         kps[0:QK_NOPE, :])
            for t4 in range(4):
                vps = ps.tile([128, 512], F32, tag="a3", name=f"vps{kb}_{t4}")
                for c in range(2):
                    nc.tensor.matmul(
                        vps[:, 0:NHL * V_HEAD],
                        cks[c][:, 128 * t4:128 * t4 + 128].bitcast(F32R),
                        wkvv_sb[:, c, :].bitcast(F32R),
                        start=(c == 0), stop=(c == 1))
                kt = 4 * kb + t4
                rot_copy(
                    Vp[:, kt].rearrange("p (h x) -> p h x", h=NHL)
                    [:, :, 64:128].bitcast(F32R),
                    vps[:, 0:NHL * V_HEAD]
                    .rearrange("p (h x) -> p h x", h=NHL))

        # ================= PHASE C =================
        # roped k_pe rows into K^T (same for all heads)
        for h in range(NHL):
            (nc.sync if h % 2 == 0 else nc.scalar).dma_start(
                out=KT[h][QK_NOPE:Q_HEAD, :].rearrange("p (r c) -> p r c", r=M),
                in_=agv_q[QLR:QLR + QK_ROPE, :, :])

        LATs = {}
        QTs = {}
        aTs = {}
        osb_state = {}

        def make_lat(qb):
            latt = sbc.tile([128, 6, 2, RB], BF16, tag="latC", bufs=2,
                            name=f"latt{qb}")
            for r in range(2):
                (nc.scalar if r == 0 else nc.sync).dma_start(
                    out=latt[:, :, r, :],
                    in_=agv_q[0:QLR].rearrange("(d p) r c -> p d r c", p=128)
                        [:, :, 2 * qb + r, :])
            LATs[qb] = latt
            QTs[qb] = {}

        def qt_chunk(qb, h):
            qs = slice(512 * qb, 512 * qb + 512)
            latt = LATs[qb]
            qps = ps.tile([128, 512], F32, tag="a2", name=f"qps{qb}_{h}")
            for c in range(6):
                nc.tensor.matmul(qps, wqb_sb[:, c, 128 * h:128 * h + 128],
                                 latt[:, c].rearrange("p r c -> p (r c)"),
                                 start=(c == 0), stop=(c == 5),
                                 skip_group_check=True)
            qt = sbc.tile([128, 512], BF16, tag="QT", bufs=10,
                          name=f"qt{qb}_{h}")
            nc.scalar.copy(qt[0:QK_NOPE, :], qps[0:QK_NOPE, :])
            t1 = sbc.tile([QK_ROPE, 512], F32, tag="rp1", bufs=2,
                          name=f"rp1_{qb}_{h}")
            nc.vector.tensor_mul(t1, qps[64:96, :], csT[0:32, qs])
            t2 = sbc.tile([QK_ROPE, 512], F32, tag="rp2", bufs=2,
                          name=f"rp2_{qb}_{h}")
            nc.vector.tensor_mul(t2, qps[96:128, :], csT[32:64, qs])
            nc.gpsimd.tensor_add(qt[QK_NOPE:Q_HEAD, :], t1, t2)
            QTs[qb][h] = qt

        def alloc_aT(qb):
            aT01 = sbc.tile([128, 512], F32, tag="aT01", bufs=2,
                            name=f"aT01_{qb}")
            aT23 = sbc.tile([128, 512], F32, tag="aT23", bufs=2,
                            name=f"aT23_{qb}")
            aT4 = sbc.tile([QK_NOPE, 512], F32, tag="aT4", bufs=2,
                           name=f"aT4_{qb}")
            aTs[qb] = (aT01, aT23, aT4)

        def wo_chunk(qb, hc, qt4):
            aT01, aT23, aT4 = aTs[qb]
            qsl = slice(128 * qt4, 128 * qt4 + 128)
            hcs = slice(512 * hc, 512 * hc + 512)
            ops = ps.tile([128, 512], F32, tag="a3", name=f"ops{qb}_{hc}_{qt4}")
            nc.tensor.matmul(ops, aT01[:, qsl].bitcast(F32R),
                             w01_sb[:, 0, hcs].bitcast(F32R),
                             start=True, stop=False, skip_group_check=True)
            nc.tensor.matmul(ops, aT23[:, qsl].bitcast(F32R),
                             w01_sb[:, 1, hcs].bitcast(F32R),
                             start=False, stop=False, skip_group_check=True)
            nc.tensor.matmul(ops, aT4[:, qsl].bitcast(F32R),
                             w2_sb[:, hcs].bitcast(F32R),
                             start=False, stop=True, skip_group_check=True)
            half, qq = divmod(qt4, 2)
            if qq == 0:
                osb_state[(qb, hc, half)] = sbc.tile(
                    [128, 2, 512], F32, tag="osb", bufs=3,
                    name=f"osb{qb}_{hc}_{half}")
            osb = osb_state[(qb, hc, half)]
            if qt4 % 2 == 0:
                nc.vector.tensor_copy(osb[:, qq, :], ops)
            else:
                nc.scalar.copy(osb[:, qq, :], ops)
            if qq == 1:
                (nc.sync if hc % 2 == 0 else nc.gpsimd).dma_start(
                    out=out_p.rearrange("(d p) c -> p d c", p=128)
                             [:, 4 * qb + 2 * half:4 * qb + 2 * half + 2, hcs],
                    in_=osb)

        def attn_head(qb, h, fills):
            nkt = 4 * qb + 4
            att = ps.tile([128, 512], F32, tag=("a0" if h % 2 == 0 else "a1"),
                          name=f"att{qb}_{h}")
            QT = QTs[qb][h]

            def emit_pv(state):
                pt2, kt0, kt1, o0, o1 = state
                for kt, o, base in ((kt0, o0, 0), (kt1, o1, 512)):
                    nc.tensor.matmul(att[:, o:512],
                                     Vp[:, kt, 128 * h:128 * h + 128]
                                     .bitcast(F32R),
                                     pt2[:, base + o:base + 512].bitcast(F32R),
                                     start=(kt == 0), stop=(kt == nkt - 1),
                                     skip_group_check=True)

            prev = None
            for p in range(nkt // 2):
                kt0, kt1 = 2 * p, 2 * p + 1
                o0 = max(0, 128 * kt0 - 512 * qb)
                o1 = max(0, 128 * kt1 - 512 * qb)
                sps = ps.tile([128, 1024], F32, tag="pr", bufs=2,
                              name=f"sps{qb}_{h}_{p}")
                nc.tensor.matmul(sps[:, o0:512],
                                 KT[h][0:Q_HEAD, 128 * kt0:128 * kt0 + 128],
                                 QT[0:Q_HEAD, o0:512],
                                 start=True, stop=True, skip_group_check=True)
                nc.tensor.matmul(sps[:, 512 + o1:1024],
                                 KT[h][0:Q_HEAD, 128 * kt1:128 * kt1 + 128],
                                 QT[0:Q_HEAD, o1:512],
                                 start=True, stop=True, skip_group_check=True)
                pt2 = sbc.tile([128, 1024], F32, tag="pt", bufs=3,
                               name=f"pt{qb}_{h}_{p}")
                nc.scalar.activation(pt2[:, o0:1024].bitcast(F32R),
                                     sps[:, o0:1024], AF.Exp, scale=SM_SCALE)
                if 128 * kt0 >= 512 * qb:   # diagonal pair
                    nc.gpsimd.tensor_mul(pt2[:, o0:o0 + 128].bitcast(F32R),
                                         pt2[:, o0:o0 + 128], tri_sb)
                    nc.gpsimd.tensor_mul(
                        pt2[:, 512 + o1:512 + o1 + 128].bitcast(F32R),
                        pt2[:, 512 + o1:512 + o1 + 128], tri_sb)
                if prev is not None:
                    emit_pv(prev)
                if fills:
                    fills.pop(0)()
                prev = (pt2, kt0, kt1, o0, o1)
            emit_pv(prev)
            # softmax division: denom is replicated on partitions 64:128
            aT01, aT23, aT4 = aTs[qb]
            if h < 2:
                dst = aT01[64 * h:64 * h + 64, :]
            elif h < 4:
                dst = aT23[64 * (h - 2):64 * (h - 2) + 64, :]
            else:
                dst = aT4
            # window is [ones|V]: denominator rows 0:64, attention 64:128
            rd = sbc.tile([QK_NOPE, 512], F32, tag="rd", bufs=2,
                          name=f"rd{qb}_{h}")
            nc.vector.reciprocal_approx_fast(rd, att[0:64, :])
            nc.vector.tensor_mul(dst.bitcast(F32R), att[64:128, :], rd)

        make_lat(0)
        for h in range(NHL):
            qt_chunk(0, h)
        for qb in range(NQB):
            if qb + 1 < NQB:
                make_lat(qb + 1)
            alloc_aT(qb)
            fills = []
            if qb + 1 < NQB:
                fills += [partial(qt_chunk, qb + 1, h) for h in range(NHL)]
            if qb > 0:
                fills += [partial(wo_chunk, qb - 1, hc, qt4)
                          for hc in range(5) for qt4 in range(4)]
            for h in range(NHL):
                attn_head(qb, h, fills)
            for f in fills:
                f()
        for hc in range(5):
            for qt4 in range(4):
                wo_chunk(NQB - 1, hc, qt4)

    nc.compile()
    return nc


def _prep(inputs):
    import ml_dtypes
    hs = np.ascontiguousarray(np.asarray(inputs["hidden_states"], np.float32)[0])
    cos = np.asarray(inputs["cos"], np.float32)
    sin = np.asarray(inputs["sin"], np.float32)
    wq_a = np.asarray(inputs["wq_a"], np.float32)
    q_ln = np.asarray(inputs["q_a_ln_w"], np.float32)
    wq_b = np.asarray(inputs["wq_b"], np.float32)
    wkv_a = np.asarray(inputs["wkv_a"], np.float32)
    kv_ln = np.asarray(inputs["kv_a_ln_w"], np.float32)
    wkv_b = np.asarray(inputs["wkv_b"], np.float32)
    wo = np.asarray(inputs["wo"], np.float32)

    if not np.all(q_ln == 1.0):
        wq_b = wq_b * q_ln[:, None]
    if not np.all(kv_ln == 1.0):
        wkv_b = wkv_b * kv_ln[:, None]

    ssin = np.concatenate([-sin[:, :16], sin[:, 16:]], axis=1)
    cosT = np.ascontiguousarray(cos.T)
    ssinT = np.ascontiguousarray(ssin.T)
    tri = np.triu(np.ones((128, 128), np.float32))
    wqa_bf = np.ascontiguousarray(wq_a).astype(ml_dtypes.bfloat16)
    wkva_bf = np.ascontiguousarray(wkv_a).astype(ml_dtypes.bfloat16)

    in_maps = []
    for c in range(M):
        heads = range(NHL * c, NHL * c + NHL)
        qb_cols = []
        for h in heads:
            qb_cols.extend(range(96 * h, 96 * h + 96))
            # swapped pe columns: [16:32] then [0:16] of the pe block
            qb_cols.extend(range(96 * h + 80, 96 * h + 96))
            qb_cols.extend(range(96 * h + 64, 96 * h + 80))
        wqb_loc = np.ascontiguousarray(wq_b[:, qb_cols]).astype(ml_dtypes.bfloat16)
        kcols, vcols = [], []
        for h in heads:
            kcols.extend(range(128 * h, 128 * h + 64))
            vcols.extend(range(128 * h + 64, 128 * h + 128))
        in_maps.append({
            "hsT": np.ascontiguousarray(
                hs[RB * c:RB * c + RB].T).astype(ml_dtypes.bfloat16),
            "cosb": np.ascontiguousarray(cos[RB * c:RB * c + RB]),
            "ssinb": np.ascontiguousarray(ssin[RB * c:RB * c + RB]),
            "cosT": cosT,
            "ssinT": ssinT,
            "tri": tri,
            "wqa": wqa_bf,
            "wkva": wkva_bf,
            "wqb_l": wqb_loc,
            "wkvk_l": np.ascontiguousarray(wkv_b[:, kcols]),
            "wkvv_l": np.ascontiguousarray(wkv_b[:, vcols]),
            "wo_l": np.ascontiguousarray(wo[NHL * V_HEAD * c:NHL * V_HEAD * (c + 1)]),
        })
    return in_maps


def kernel(**inputs):
    if "nc" not in _cache:
        _cache["nc"] = _build()
    nc = _cache["nc"]
    in_maps = _prep(inputs)
    res = run_bass_kernel_spmd(nc, in_maps, core_ids=list(range(M)))
    out = res.results[0]["out_p"].astype(np.float32)
    for c in range(1, M):
        out += res.results[c]["out_p"]
    return out.reshape(1, S, H)


# revision 13
# speedup vs baseline: 1.3393x; 1.0000x over previous
"""MiniCPM (MLA-style) attention — Trainium2 Bass kernel, 8-way sharded.

Strategy (tensor-parallel over heads, 5 heads/core; seq-parallel low-rank
phase A with AllGather of the latents):

  Phase A (seq-parallel, 256 rows/core): hs^T arrives pre-transposed from
  the host (bf16); qa/ckv computed in natural layout via bf16 matmuls with
  all weights bulk-DMAed up front; rms_norm; outputs transposed on-chip and
  AllGathered (kv latents f32, q latents + roped k_pe bf16).

  Phase B: K^T (bf16) and V' (f32) built per head from the gathered
  latents.  V' layout per k-tile: [V_h0|..|V_h4|ones64] — the 64 ones
  columns make every PV matmul emit the softmax denominator replicated
  across partitions 64:128 of the attention accumulator, so the epilogue
  is a single DVE divide (no reciprocal / partition-broadcast chain).

  Phase C (per q-block of 512, per head sequentially): scores S^T[k,q]
  computed two k-tiles at a time into a 2-bank PSUM pair, one ACT exp per
  pair (halves ACT instruction overhead), causal tri-mask on DVE, PV
  accumulation with the ones-block stationary.  wo matmuls of the previous
  q-block and Q^T builds of the next q-block are interleaved as PE filler
  so the PE never idles long enough for the HAM clock-gate to re-throttle.

  wo: resident in SBUF; each core computes a full [2048,2560] partial with
  its 320 rows; host sums the 8 partials.
"""

import sys
sys.path.insert(0, "/opt/trn_rl_repo")

from contextlib import ExitStack
from functools import partial

import numpy as np

import concourse.bass as bass
import concourse.bacc as bacc
import concourse.tile as tile
from concourse import mybir
from concourse.bass_utils import run_bass_kernel_spmd
from concourse.masks import make_identity

F32 = mybir.dt.float32
F32R = mybir.dt.float32r
BF16 = mybir.dt.bfloat16
AF = mybir.ActivationFunctionType
ALU = mybir.AluOpType

M = 8                  # cores
S = 2048               # sequence
H = 2560               # hidden
RB = S // M            # 256 rows per core (phase A)
QLR = 768              # q low rank
CKV = 256              # kv low rank (normed part)
QK_ROPE = 32
QK_NOPE = 64
Q_HEAD = 96
V_HEAD = 64
NH = 40
NHL = NH // M          # 5 heads per core
EPS = 1e-6
SM_SCALE = float(Q_HEAD) ** -0.5
NQB = S // 512         # 4 q-blocks
NKT = S // 128         # 16 k-tiles
# V' per k-tile: [ones|V0 | ones|V1 | ... | ones|V4] — every head h reads a
# CONTIGUOUS 128-col stationary window [ones64|V_h] at 128h, so the softmax
# denominator always lands partition-0-aligned in the attention accumulator
# (reciprocal_approx_fast requires a partition-0-based input).
VROW = 640
HC = H // 128              # 20 hidden-dim k-tiles

_cache = {}


def _build():
    nc = bacc.Bacc(trn_type="TRN2", target_bir_lowering=False, debug=False,
                   num_devices=M)

    # ---- I/O ----
    hsT_d = nc.dram_tensor("hsT", [H, RB], BF16, kind="ExternalInput").ap()
    wqa_d = nc.dram_tensor("wqa", [H, QLR], BF16, kind="ExternalInput").ap()
    wkva_d = nc.dram_tensor("wkva", [H, CKV + QK_ROPE], BF16,
                            kind="ExternalInput").ap()
    cosb = nc.dram_tensor("cosb", [RB, QK_ROPE], F32, kind="ExternalInput").ap()
    ssinb = nc.dram_tensor("ssinb", [RB, QK_ROPE], F32, kind="ExternalInput").ap()
    cosT = nc.dram_tensor("cosT", [QK_ROPE, S], F32, kind="ExternalInput").ap()
    ssinT = nc.dram_tensor("ssinT", [QK_ROPE, S], F32, kind="ExternalInput").ap()
    tri = nc.dram_tensor("tri", [128, 128], F32, kind="ExternalInput").ap()
    wqb_l = nc.dram_tensor("wqb_l", [QLR, NHL * 128], BF16,
                           kind="ExternalInput").ap()
    wkvk_l = nc.dram_tensor("wkvk_l", [CKV, NHL * QK_NOPE], F32,
                            kind="ExternalInput").ap()
    wkvv_l = nc.dram_tensor("wkvv_l", [CKV, NHL * V_HEAD], F32,
                            kind="ExternalInput").ap()
    wo_l = nc.dram_tensor("wo_l", [NHL * V_HEAD, H], F32,
                          kind="ExternalInput").ap()
    out_p = nc.dram_tensor("out_p", [S, H], F32, kind="ExternalOutput").ap()

    agin_kv = nc.dram_tensor("agin_kv", [CKV, RB], F32, kind="Internal").ap()
    agout_kv = nc.dram_tensor("agout_kv", [M * CKV, RB], F32,
                              kind="Internal", addr_space="Shared").ap()
    agin_q = nc.dram_tensor("agin_q", [QLR + QK_ROPE, RB], BF16,
                            kind="Internal").ap()
    agout_q = nc.dram_tensor("agout_q", [M * (QLR + QK_ROPE), RB], BF16,
                             kind="Internal", addr_space="Shared").ap()
    agv_kv = agout_kv.rearrange("(r n) c -> n r c", r=M)   # [256, 8, 256]
    agv_q = agout_q.rearrange("(r n) c -> n r c", r=M)     # [800, 8, 256]

    with ExitStack() as ctx:
        tc = ctx.enter_context(tile.TileContext(nc))

        const = ctx.enter_context(tc.tile_pool(name="const", bufs=1))
        persist = ctx.enter_context(tc.tile_pool(name="persist", bufs=1))
        ps = ctx.enter_context(tc.tile_pool(name="ps", bufs=1, space="PSUM"))
        actx = ExitStack()
        sba = actx.enter_context(tc.tile_pool(name="sba", bufs=1))

        # ---- upfront DMAs: phase-A criticals first ----
        hsT_sb = sba.tile([128, HC, RB], BF16)
        nc.sync.dma_start(out=hsT_sb, in_=hsT_d.rearrange("(t p) c -> p t c", p=128))
        wkva_sb = sba.tile([128, HC, CKV + QK_ROPE], BF16)
        nc.sync.dma_start(out=wkva_sb,
                          in_=wkva_d.rearrange("(t p) c -> p t c", p=128))
        wqa_sb = sba.tile([128, HC, QLR], BF16)
        nc.sync.dma_start(out=wqa_sb,
                          in_=wqa_d.rearrange("(t p) c -> p t c", p=128))
        wqb_sb = const.tile([128, 6, NHL * 128], BF16)
        nc.sync.dma_start(out=wqb_sb,
                          in_=wqb_l.rearrange("(t p) c -> p t c", p=128))

        ident = const.tile([128, 128], F32)
        make_identity(nc, ident)
        tri_sb = const.tile([128, 128], F32)
        nc.gpsimd.dma_start(out=tri_sb, in_=tri)
        eps_t = const.tile([128, 1], F32)
        nc.vector.memset(eps_t, EPS)
        # packed cos/sin (transposed) [64, 2048]: rows 0:32 cosT, 32:64 ssinT
        csT = const.tile([64, S], F32)
        nc.gpsimd.dma_start(out=csT[0:32, :], in_=cosT)
        nc.gpsimd.dma_start(out=csT[32:64, :], in_=ssinT)
        # natural-block cos/ssin [128, 2, 64]
        csb = const.tile([128, 2, 2 * QK_ROPE], F32)
        nc.gpsimd.dma_start(out=csb[:, :, 0:QK_ROPE],
                            in_=cosb.rearrange("(t p) c -> p t c", p=128))
        nc.gpsimd.dma_start(out=csb[:, :, QK_ROPE:],
                            in_=ssinb.rearrange("(t p) c -> p t c", p=128))
        wkvk_sb = const.tile([128, 2, NHL * QK_NOPE], F32)
        nc.gpsimd.dma_start(out=wkvk_sb.bitcast(F32R),
                            in_=wkvk_l.rearrange("(t p) c -> p t c", p=128).bitcast(F32R))
        wkvv_sb = const.tile([128, 2, NHL * V_HEAD], F32)
        nc.gpsimd.dma_start(out=wkvv_sb.bitcast(F32R),
                            in_=wkvv_l.rearrange("(t p) c -> p t c", p=128).bitcast(F32R))
        # wo resident (needed only in phase C — queued last)
        w01_sb = const.tile([128, 2, H], F32)
        nc.sync.dma_start(out=w01_sb.bitcast(F32R),
                          in_=wo_l[0:256].rearrange("(d p) c -> p d c", p=128)
                          .bitcast(F32R))
        w2_sb = const.tile([QK_NOPE, H], F32)
        nc.sync.dma_start(out=w2_sb.bitcast(F32R),
                          in_=wo_l[256:320].bitcast(F32R))

        # ---- persistent K^T / V' ----
        KT = [persist.tile([128, S], BF16, tag=f"KT{h}", name=f"KT{h}")
              for h in range(NHL)]
        Vp = persist.tile([128, NKT, VROW], F32, tag="Vp")
        nc.vector.memset(Vp, 1.0)

        # ================= PHASE A =================
        # ckv = hs @ wkv_a   (rt0 in cols 0:288, rt1 in cols 512:800)
        ckv_ps = ps.tile([128, 1024], F32, tag="pr", bufs=2, name="ckv_ps")
        ckv_v = [ckv_ps[:, 0:CKV + QK_ROPE],
                 ckv_ps[:, 512:512 + CKV + QK_ROPE]]
        for hc in range(HC):
            for rt in range(2):
                nc.tensor.matmul(ckv_v[rt],
                                 hsT_sb[:, hc, 128 * rt:128 * rt + 128],
                                 wkva_sb[:, hc, :],
                                 start=(hc == 0), stop=(hc == HC - 1))
        for rt in range(2):
            sq3 = sba.tile([128, CKV], F32, tag="sq", name=f"sq3_{rt}")
            ac = sba.tile([128, 1], F32, tag="st0", name=f"ac_{rt}")
            nc.scalar.activation(sq3, ckv_v[rt][:, 0:CKV], AF.Square, accum_out=ac)
            nc.scalar.activation(ac, ac, AF.Sqrt, bias=eps_t, scale=1.0 / CKV)
            crstd = sba.tile([128, 1], F32, tag="st1", name=f"crstd_{rt}")
            nc.vector.reciprocal_approx_fast(crstd, ac)
            ckvn = sba.tile([128, CKV], F32, tag="ckvn", bufs=2, name=f"ckvn_{rt}")
            nc.vector.tensor_scalar_mul(ckvn, ckv_v[rt][:, 0:CKV], crstd)
            for jc in range(2):
                tp = ps.tile([128, 128], F32, tag=("a2" if jc == 0 else "a3"),
                             name=f"tpc_{rt}_{jc}")
                nc.tensor.transpose(tp, ckvn[:, 128 * jc:128 * jc + 128], ident)
                pc = sba.tile([128, 128], F32, tag="pc", bufs=4,
                              name=f"pcc_{rt}_{jc}")
                if jc == 0:
                    nc.vector.tensor_copy(pc, tp)
                else:
                    nc.scalar.copy(pc, tp)
                nc.scalar.dma_start(out=agin_kv[128 * jc:128 * jc + 128,
                                                128 * rt:128 * rt + 128], in_=pc)

        nc.gpsimd.collective_compute(
            "AllGather", mybir.AluOpType.bypass,
            replica_groups=[list(range(M))],
            ins=[agin_kv], outs=[agout_kv],
        )

        # k_pe RoPE (natural layout) then transpose, into agin_q rows 768:800
        for rt in range(2):
            t1 = sba.tile([128, QK_ROPE], F32, tag="kp1", name=f"kp1_{rt}")
            nc.vector.tensor_mul(t1, ckv_v[rt][:, CKV:CKV + QK_ROPE],
                                 csb[:, rt, 0:QK_ROPE])
            t2 = sba.tile([128, QK_ROPE], F32, tag="kp2", name=f"kp2_{rt}")
            nc.vector.tensor_mul(t2[:, 0:16], ckv_v[rt][:, CKV + 16:CKV + 32],
                                 csb[:, rt, QK_ROPE:QK_ROPE + 16])
            nc.vector.tensor_mul(t2[:, 16:32], ckv_v[rt][:, CKV:CKV + 16],
                                 csb[:, rt, QK_ROPE + 16:QK_ROPE + 32])
            kpe = sba.tile([128, QK_ROPE], F32, tag="kp3", name=f"kp3_{rt}")
            nc.vector.tensor_add(kpe, t1, t2)
            tp = ps.tile([128, 128], F32, tag="a2", name=f"tpk_{rt}")
            nc.tensor.transpose(tp[0:QK_ROPE, :], kpe, ident)
            pck = sba.tile([QK_ROPE, 128], BF16, tag="pck", bufs=2,
                           name=f"pck_{rt}")
            nc.vector.tensor_copy(pck, tp[0:QK_ROPE, :])
            nc.scalar.dma_start(out=agin_q[QLR:QLR + QK_ROPE,
                                           128 * rt:128 * rt + 128], in_=pck)

        # qa = hs @ wq_a  (rt0 in pr slot halves; rt1 in a0/a1)
        qa0 = ps.tile([128, 1024], F32, tag="pr", bufs=2, name="qa0")
        qa1a = ps.tile([128, 384], F32, tag="a0", name="qa1a")
        qa1b = ps.tile([128, 384], F32, tag="a1", name="qa1b")
        qa_v = [[qa0[:, 0:384], qa0[:, 512:896]], [qa1a, qa1b]]
        for hc in range(HC):
            for rt in range(2):
                for jt in range(2):
                    nc.tensor.matmul(qa_v[rt][jt],
                                     hsT_sb[:, hc, 128 * rt:128 * rt + 128],
                                     wqa_sb[:, hc, 384 * jt:384 * jt + 384],
                                     start=(hc == 0), stop=(hc == HC - 1))
        for rt in range(2):
            sq = sba.tile([128, 384], F32, tag="sq", name=f"sq_{rt}")
            a0s = sba.tile([128, 1], F32, tag="st0", name=f"a0s_{rt}")
            a1s = sba.tile([128, 1], F32, tag="st1", name=f"a1s_{rt}")
            nc.scalar.activation(sq, qa_v[rt][0], AF.Square, accum_out=a0s)
            sq2 = sba.tile([128, 384], F32, tag="sq", name=f"sq2_{rt}")
            nc.scalar.activation(sq2, qa_v[rt][1], AF.Square, accum_out=a1s)
            ssum = sba.tile([128, 1], F32, tag="st2", name=f"ssum_{rt}")
            nc.vector.tensor_add(ssum, a0s, a1s)
            nc.scalar.activation(ssum, ssum, AF.Sqrt, bias=eps_t, scale=1.0 / QLR)
            rstd = sba.tile([128, 1], F32, tag="st3", name=f"rstd_{rt}")
            nc.vector.reciprocal_approx_fast(rstd, ssum)
            qan = sba.tile([128, QLR], F32, tag="qan", bufs=2, name=f"qan_{rt}")
            for jt in range(2):
                nc.vector.tensor_scalar_mul(qan[:, 384 * jt:384 * jt + 384],
                                            qa_v[rt][jt], rstd)
            for jc in range(6):
                tp = ps.tile([128, 128], F32, tag=("a2" if jc % 2 == 0 else "a3"),
                             name=f"tpq_{rt}_{jc}")
                nc.tensor.transpose(tp, qan[:, 128 * jc:128 * jc + 128], ident)
                pcq = sba.tile([128, 128], BF16, tag="pcb", bufs=4,
                               name=f"pcq_{rt}_{jc}")
                if jc % 2 == 0:
                    nc.vector.tensor_copy(pcq, tp)
                else:
                    nc.scalar.copy(pcq, tp)
                nc.scalar.dma_start(
                    out=agin_q[128 * jc:128 * jc + 128,
                               128 * rt:128 * rt + 128], in_=pcq)

        nc.gpsimd.collective_compute(
            "AllGather", mybir.AluOpType.bypass,
            replica_groups=[list(range(M))],
            ins=[agin_q], outs=[agout_q],
        )
        actx.close()
        sbc = ctx.enter_context(tc.tile_pool(name="sbc", bufs=1))

        # ================= PHASE B: K^T and V' =================
        cp_engines = [nc.vector, nc.scalar]
        cpi = [0]

        def rot_copy(dst, src):
            e = cp_engines[cpi[0] % 2]
            cpi[0] += 1
            if e is nc.scalar:
                e.copy(dst, src)
            else:
                e.tensor_copy(dst, src)

        for kb in range(NQB):
            ckt = sbc.tile([128, 2, 2, RB], F32, tag="ckt", bufs=2,
                           name=f"ckt{kb}")
            for d in range(2):
                nc.sync.dma_start(
                    out=ckt[:, d].bitcast(F32R),
                    in_=agv_kv[128 * d:128 * d + 128,
                               2 * kb:2 * kb + 2, :].bitcast(F32R))
            cks = [ckt[:, c].rearrange("p r c -> p (r c)") for c in range(2)]
            for h in range(NHL):
                kps = ps.tile([128, 512], F32, tag="a2", name=f"kps{kb}_{h}")
                for c in range(2):
                    nc.tensor.matmul(
                        kps[0:QK_NOPE, :],
                        wkvk_sb[:, c, QK_NOPE * h:QK_NOPE * h + QK_NOPE]
                        .bitcast(F32R),
                        cks[c].bitcast(F32R),
                        start=(c == 0), stop=(c == 1))
                rot_copy(KT[h][0:QK_NOPE, 512 * kb:512 * kb + 512],
                         kps[0:QK_NOPE, :])
            for t4 in range(4):
                vps = ps.tile([128, 512], F32, tag="a3", name=f"vps{kb}_{t4}")
                for c in range(2):
                    nc.tensor.matmul(
                        vps[:, 0:NHL * V_HEAD],
                        cks[c][:, 128 * t4:128 * t4 + 128].bitcast(F32R),
                        wkvv_sb[:, c, :].bitcast(F32R),
                        start=(c == 0), stop=(c == 1))
                kt = 4 * kb + t4
                rot_copy(
                    Vp[:, kt].rearrange("p (h x) -> p h x", h=NHL)
                    [:, :, 64:128].bitcast(F32R),
                    vps[:, 0:NHL * V_HEAD]
                    .rearrange("p (h x) -> p h x", h=NHL))

        # ================= PHASE C =================
        # roped k_pe rows into K^T (same for all heads)
        for h in range(NHL):
            (nc.sync if h % 2 == 0 else nc.scalar).dma_start(
                out=KT[h][QK_NOPE:Q_HEAD, :].rearrange("p (r c) -> p r c", r=M),
                in_=agv_q[QLR:QLR + QK_ROPE, :, :])

        LATs = {}
        QTs = {}
        aTs = {}
        osb_state = {}

        def make_lat(qb):
            latt = sbc.tile([128, 6, 2, RB], BF16, tag="latC", bufs=2,
                            name=f"latt{qb}")
            for r in range(2):
                (nc.scalar if r == 0 else nc.sync).dma_start(
                    out=latt[:, :, r, :],
                    in_=agv_q[0:QLR].rearrange("(d p) r c -> p d r c", p=128)
                        [:, :, 2 * qb + r, :])
            LATs[qb] = latt
            QTs[qb] = {}

        def qt_chunk(qb, h):
            qs = slice(512 * qb, 512 * qb + 512)
            latt = LATs[qb]
            qps = ps.tile([128, 512], F32, tag="a2", name=f"qps{qb}_{h}")
            for c in range(6):
                nc.tensor.matmul(qps, wqb_sb[:, c, 128 * h:128 * h + 128],
                                 latt[:, c].rearrange("p r c -> p (r c)"),
                                 start=(c == 0), stop=(c == 5),
                                 skip_group_check=True)
            qt = sbc.tile([128, 512], BF16, tag="QT", bufs=10,
                          name=f"qt{qb}_{h}")
            nc.scalar.copy(qt[0:QK_NOPE, :], qps[0:QK_NOPE, :])
            t1 = sbc.tile([QK_ROPE, 512], F32, tag="rp1", bufs=2,
                          name=f"rp1_{qb}_{h}")
            nc.vector.tensor_mul(t1, qps[64:96, :], csT[0:32, qs])
            t2 = sbc.tile([QK_ROPE, 512], F32, tag="rp2", bufs=2,
                          name=f"rp2_{qb}_{h}")
            nc.vector.tensor_mul(t2, qps[96:128, :], csT[32:64, qs])
            nc.gpsimd.tensor_add(qt[QK_NOPE:Q_HEAD, :], t1, t2)
            QTs[qb][h] = qt

        def alloc_aT(qb):
            aT01 = sbc.tile([128, 512], F32, tag="aT01", bufs=2,
                            name=f"aT01_{qb}")
            aT23 = sbc.tile([128, 512], F32, tag="aT23", bufs=2,
                            name=f"aT23_{qb}")
            aT4 = sbc.tile([QK_NOPE, 512], F32, tag="aT4", bufs=2,
                           name=f"aT4_{qb}")
            aTs[qb] = (aT01, aT23, aT4)

        def wo_chunk(qb, hc, qt4):
            aT01, aT23, aT4 = aTs[qb]
            qsl = slice(128 * qt4, 128 * qt4 + 128)
            hcs = slice(512 * hc, 512 * hc + 512)
            ops = ps.tile([128, 512], F32, tag="a3", name=f"ops{qb}_{hc}_{qt4}")
            nc.tensor.matmul(ops, aT01[:, qsl].bitcast(F32R),
                             w01_sb[:, 0, hcs].bitcast(F32R),
                             start=True, stop=False, skip_group_check=True)
            nc.tensor.matmul(ops, aT23[:, qsl].bitcast(F32R),
                             w01_sb[:, 1, hcs].bitcast(F32R),
                             start=False, stop=False, skip_group_check=True)
            nc.tensor.matmul(ops, aT4[:, qsl].bitcast(F32R),
                             w2_sb[:, hcs].bitcast(F32R),
                             start=False, stop=True, skip_group_check=True)
            half, qq = divmod(qt4, 2)
            if qq == 0:
                osb_state[(qb, hc, half)] = sbc.tile(
                    [128, 2, 512], F32, tag="osb", bufs=3,
                    name=f"osb{qb}_{hc}_{half}")
            osb = osb_state[(qb, hc, half)]
            if qt4 % 2 == 0:
                nc.vector.tensor_copy(osb[:, qq, :], ops)
            else:
                nc.scalar.copy(osb[:, qq, :], ops)
            if qq == 1:
                (nc.sync if hc % 2 == 0 else nc.gpsimd).dma_start(
                    out=out_p.rearrange("(d p) c -> p d c", p=128)
                             [:, 4 * qb + 2 * half:4 * qb + 2 * half + 2, hcs],
                    in_=osb)

        def attn_head(qb, h, fills):
            nkt = 4 * qb + 4
            att = ps.tile([128, 512], F32, tag=("a0" if h % 2 == 0 else "a1"),
                          name=f"att{qb}_{h}")
            QT = QTs[qb][h]

            def emit_pv(state):
                pt2, kt0, kt1, o0, o1 = state
                for kt, o, base in ((kt0, o0, 0), (kt1, o1, 512)):
                    nc.tensor.matmul(att[:, o:512],
                                     Vp[:, kt, 128 * h:128 * h + 128]
                                     .bitcast(F32R),
                                     pt2[:, base + o:base + 512].bitcast(F32R),
                                     start=(kt == 0), stop=(kt == nkt - 1),
                                     skip_group_check=True)

            prev = None
            for p in range(nkt // 2):
                kt0, kt1 = 2 * p, 2 * p + 1
                o0 = max(0, 128 * kt0 - 512 * qb)
                o1 = max(0, 128 * kt1 - 512 * qb)
                sps = ps.tile([128, 1024], F32, tag="pr", bufs=2,
                              name=f"sps{qb}_{h}_{p}")
                nc.tensor.matmul(sps[:, o0:512],
                                 KT[h][0:Q_HEAD, 128 * kt0:128 * kt0 + 128],
                                 QT[0:Q_HEAD, o0:512],
                                 start=True, stop=True, skip_group_check=True)
                nc.tensor.matmul(sps[:, 512 + o1:1024],
                                 KT[h][0:Q_HEAD, 128 * kt1:128 * kt1 + 128],
                                 QT[0:Q_HEAD, o1:512],
                                 start=True, stop=True, skip_group_check=True)
                pt2 = sbc.tile([128, 1024], F32, tag="pt", bufs=3,
                               name=f"pt{qb}_{h}_{p}")
                nc.scalar.activation(pt2[:, o0:1024].bitcast(F32R),
                                     sps[:, o0:1024], AF.Exp, scale=SM_SCALE)
                if 128 * kt0 >= 512 * qb:   # diagonal pair
                    nc.gpsimd.tensor_mul(pt2[:, o0:o0 + 128].bitcast(F32R),
                                         pt2[:, o0:o0 + 128], tri_sb)
                    nc.gpsimd.tensor_mul(
                        pt2[:, 512 + o1:512 + o1 + 128].bitcast(F32R),
                        pt2[:, 512 + o1:512 + o1 + 128], tri_sb)
                if prev is not None:
                    emit_pv(prev)
                if fills:
                    fills.pop(0)()
                prev = (pt2, kt0, kt1, o0, o1)
            emit_pv(prev)
            # softmax division: denom is replicated on partitions 64:128
            aT01, aT23, aT4 = aTs[qb]
            if h < 2:
                dst = aT01[64 * h:64 * h + 64, :]
            elif h < 4:
                dst = aT23[64 * (h - 2):64 * (h - 2) + 64, :]
            else:
                dst = aT4
            # window is [ones|V]: denominator rows 0:64, attention 64:128
            rd = sbc.tile([QK_NOPE, 512], F32, tag="rd", bufs=2,
                          name=f"rd{qb}_{h}")
            nc.vector.reciprocal_approx_fast(rd, att[0:64, :])
            nc.vector.tensor_mul(dst.bitcast(F32R), att[64:128, :], rd)

        make_lat(0)
        for h in range(NHL):
            qt_chunk(0, h)
        deferred = []
        for qb in range(NQB):
            if qb + 1 < NQB:
                make_lat(qb + 1)
            alloc_aT(qb)
            fills = []
            if qb + 1 < NQB:
                fills += [partial(qt_chunk, qb + 1, h) for h in range(NHL)]
            if qb > 0:
                wo_prev = [partial(wo_chunk, qb - 1, hc, qt4)
                           for hc in range(5) for qt4 in range(4)]
                if qb == 1:
                    fills += wo_prev[:15]
                    deferred.extend(wo_prev[15:])
                else:
                    fills += deferred
                    deferred.clear()
                    fills += wo_prev
            for h in range(NHL):
                attn_head(qb, h, fills)
            for f in fills:
                f()
        for hc in range(5):
            for qt4 in range(4):
                wo_chunk(NQB - 1, hc, qt4)

    nc.compile()
    return nc


def _prep(inputs):
    import ml_dtypes
    hs = np.ascontiguousarray(np.asarray(inputs["hidden_states"], np.float32)[0])
    cos = np.asarray(inputs["cos"], np.float32)
    sin = np.asarray(inputs["sin"], np.float32)
    wq_a = np.asarray(inputs["wq_a"], np.float32)
    q_ln = np.asarray(inputs["q_a_ln_w"], np.float32)
    wq_b = np.asarray(inputs["wq_b"], np.float32)
    wkv_a = np.asarray(inputs["wkv_a"], np.float32)
    kv_ln = np.asarray(inputs["kv_a_ln_w"], np.float32)
    wkv_b = np.asarray(inputs["wkv_b"], np.float32)
    wo = np.asarray(inputs["wo"], np.float32)

    if not np.all(q_ln == 1.0):
        wq_b = wq_b * q_ln[:, None]
    if not np.all(kv_ln == 1.0):
        wkv_b = wkv_b * kv_ln[:, None]

    ssin = np.concatenate([-sin[:, :16], sin[:, 16:]], axis=1)
    cosT = np.ascontiguousarray(cos.T)
    ssinT = np.ascontiguousarray(ssin.T)
    tri = np.triu(np.ones((128, 128), np.float32))
    wqa_bf = np.ascontiguousarray(wq_a).astype(ml_dtypes.bfloat16)
    wkva_bf = np.ascontiguousarray(wkv_a).astype(ml_dtypes.bfloat16)

    in_maps = []
    for c in range(M):
        heads = range(NHL * c, NHL * c + NHL)
        qb_cols = []
        for h in heads:
            qb_cols.extend(range(96 * h, 96 * h + 96))
            # swapped pe columns: [16:32] then [0:16] of the pe block
            qb_cols.extend(range(96 * h + 80, 96 * h + 96))
            qb_cols.extend(range(96 * h + 64, 96 * h + 80))
        wqb_loc = np.ascontiguousarray(wq_b[:, qb_cols]).astype(ml_dtypes.bfloat16)
        kcols, vcols = [], []
        for h in heads:
            kcols.extend(range(128 * h, 128 * h + 64))
            vcols.extend(range(128 * h + 64, 128 * h + 128))
        in_maps.append({
            "hsT": np.ascontiguousarray(
                hs[RB * c:RB * c + RB].T).astype(ml_dtypes.bfloat16),
            "cosb": np.ascontiguousarray(cos[RB * c:RB * c + RB]),
            "ssinb": np.ascontiguousarray(ssin[RB * c:RB * c + RB]),
            "cosT": cosT,
            "ssinT": ssinT,
            "tri": tri,
            "wqa": wqa_bf,
            "wkva": wkva_bf,
            "wqb_l": wqb_loc,
            "wkvk_l": np.ascontiguousarray(wkv_b[:, kcols]),
            "wkvv_l": np.ascontiguousarray(wkv_b[:, vcols]),
            "wo_l": np.ascontiguousarray(wo[NHL * V_HEAD * c:NHL * V_HEAD * (c + 1)]),
        })
    return in_maps


def kernel(**inputs):
    if "nc" not in _cache:
        _cache["nc"] = _build()
    nc = _cache["nc"]
    in_maps = _prep(inputs)
    res = run_bass_kernel_spmd(nc, in_maps, core_ids=list(range(M)))
    out = res.results[0]["out_p"].astype(np.float32)
    for c in range(1, M):
        out += res.results[c]["out_p"]
    return out.reshape(1, S, H)
